# revision 9
# baseline (speedup 1.0000x reference)
"""CRNN greedy CTC-style decoder kernel for Trainium2 (Bass/Tile).

Problem: logits [B=2048, C=12, T=2048] f32 ->
  decoded     [B, 6] int32  (first 6 CTC-collapsed tokens, pad -1)
  confidences [B, 6] f32    (per-kept-timestep softmax entropy, pad 0)

Sharding: pure data-parallel over batch across 8 NeuronCores
(256 rows/core), no communication.

Key observation: with i.i.d. logits the keep probability per timestep is
(11/12)^2 ~ 0.84, so every row resolves its 6 output slots within the
first ~12 timesteps (measured max t = 11 for the full input).  The hot
path therefore only reads/decodes logits[:, :, 0:HEAD] (HEAD=16):

  Hot path (always runs, packed 2 rows per partition -> one pass):
    phase 1: exact argmax over C via max/one-hot/max chain (ties resolve
      to the smallest class index, bit-exact with jnp.argmax).
    phase 2: run-dedup mask, inclusive cumsum (scan) -> pos1.
    phase 3: entropy H = lnZ - (sum_c e^l * l)/Z (exact identity; the
      reference's +1e-6 inside the log only shifts H by ~1e-5 relative),
      slot extraction via one-hot (pos1==j+1 & mask) multiply + windowed
      reduce.  Work split across DVE (reduces) / Pool (elementwise) /
      Act (exp, ln) engines.

  Flag: one PE matmul counts rows with pos1[HEAD-1] < 6.  If any row is
  unresolved (statistically never; impossible for the seed-0 input), a
  guarded cold path recomputes preds/mask/pos1 over the full T and
  accumulates slot contributions from t >= HEAD, preserving worst-case
  correctness for arbitrary inputs.

Perf: ~212 us baseline (full-T argmax sweep, DVE-bound) -> head-gated
hot path is DMA-latency + a few us of tiny-tile compute.
"""

import numpy as np

import concourse.bass as bass
import concourse.bacc as bacc
import concourse.mybir as mybir
import concourse.tile as tile
from concourse.bass_utils import run_bass_kernel_spmd

F32 = mybir.dt.float32
BF16 = mybir.dt.bfloat16
I32 = mybir.dt.int32
Alu = mybir.AluOpType
Act = mybir.ActivationFunctionType
AX = mybir.AxisListType.X

N_CORES = 8
MAXLEN = 6
BLANK = 11
PAD = -1

# full problem shape (hardcoded per the harness contract)
B_FULL, C, T_FULL = 2048, 12, 2048
JW = MAXLEN
HEAD = 16


def _v(t, off, dims):
    """AP on tile t at element offset `off`: dims = [(step, count), ...]."""
    ap = t[:]
    return bass.AP(ap.tensor, ap.offset + off, [ap.ap[0]] + [list(d) for d in dims])


def build_decoder(nc, B, T, head=HEAD):
    """Emit the per-core decoder program.  B = rows per core (must be 256)."""
    assert B == 256, "hot path packs exactly 2 row-halves per partition"
    H = head
    NB = B // 128  # = 2 row-halves

    lg = nc.dram_tensor("logits", [B, C, T], F32, kind="ExternalInput")
    dec_o = nc.dram_tensor("decoded", [B, MAXLEN], I32, kind="ExternalOutput")
    conf_o = nc.dram_tensor("confidences", [B, MAXLEN], F32, kind="ExternalOutput")

    with tile.TileContext(nc) as tc:
        with (
            tc.tile_pool(name="consts", bufs=1) as consts,
            tc.tile_pool(name="hot", bufs=1) as hot,
            tc.tile_pool(name="clt", bufs=2) as clt,
            tc.tile_pool(name="ceq", bufs=2) as ceq,
            tc.tile_pool(name="cm", bufs=2) as cm,
            tc.tile_pool(name="cperbc", bufs=NB) as cperbc,
            tc.tile_pool(name="cph3", bufs=2) as cph3,
            tc.tile_pool(name="psum", bufs=1, space="PSUM") as psum_pool,
        ):
            # ---------------- constants ----------------
            # reversed class weights 11-c: argmax extracted via MAX of
            # eq*(11-c) -> smallest class index wins ties (= jnp.argmax).
            cio_i = consts.tile([128, C], I32, tag="cio_i")
            nc.gpsimd.iota(cio_i[:], pattern=[[-1, C]], base=C - 1,
                           channel_multiplier=0)
            cio = consts.tile([128, C], BF16, tag="cio")
            nc.vector.tensor_copy(cio[:], cio_i[:])

            jio_i = consts.tile([128, JW], I32, tag="jio_i")
            nc.gpsimd.iota(jio_i[:], pattern=[[1, JW]], base=1,
                           channel_multiplier=0)
            jio = consts.tile([128, JW], F32, tag="jio")
            nc.vector.tensor_copy(jio[:], jio_i[:])

            ones = consts.tile([128, 1], F32, tag="ones")
            nc.vector.memset(ones[:], 1.0)

            # ================= HOT PATH =================
            # lh layout (r, c, t): off = r*C*H + c*H + t
            lh = hot.tile([128, NB * C * H], F32, tag="lh")
            for r in range(NB):
                dst = _v(lh, r * C * H, [(H, C), (1, H)])
                src = lg[r * 128:(r + 1) * 128, :, 0:H]
                (nc.sync if r == 0 else nc.scalar).dma_start(dst, src)

            # ---- phase 1: exact argmax (DVE) ----
            # (HW BIR verifier caps APs at 3 dims (2 free): per-row-half ops
            # where a (r,t)x(c) pairing is needed; collapsed (rt) otherwise.)
            m = hot.tile([128, NB * H], F32, tag="m")
            for r in range(NB):
                nc.vector.tensor_reduce(
                    m[:, r * H:(r + 1) * H],
                    _v(lh, r * C * H, [(1, H), (H, C)]), axis=AX, op=Alu.max)

            # eq layout (r, t, c): off = (r*H + t)*C + c  (c contiguous -> 2x)
            eq = hot.tile([128, NB * H * C], BF16, tag="eq")
            for r in range(NB):
                nc.vector.scalar_tensor_tensor(
                    _v(eq, r * H * C, [(C, H), (1, C)]),
                    _v(m, r * H, [(1, H), (0, C)]), 1.0,
                    _v(lh, r * C * H, [(1, H), (H, C)]),
                    op0=Alu.mult, op1=Alu.is_le)
            w = hot.tile([128, NB * H * C], BF16, tag="w")
            eq_v = _v(eq, 0, [(C, NB * H), (1, C)])
            w_v = _v(w, 0, [(C, NB * H), (1, C)])
            cio_bv = _v(cio, 0, [(0, NB * H), (1, C)])
            nc.vector.tensor_tensor(w_v, eq_v, cio_bv, op=Alu.mult)
            # predsh[:, r*H + t] = 11 - argmax_c  (blank=11 -> 0)
            predsh = hot.tile([128, NB * H], BF16, tag="predsh")
            nc.vector.tensor_reduce(predsh[:], w_v, axis=AX, op=Alu.max)

            # ---- phase 3a: entropy inputs (Act + Pool + DVE) ----
            # e = exp(l) (no max-subtract: |l| <= ~6 is safe in f32)
            e = hot.tile([128, NB * C * H], F32, tag="e")
            nc.scalar.activation(e[:], lh[:], Act.Exp)
            el = hot.tile([128, NB * C * H], F32, tag="el")
            nc.vector.tensor_tensor(el[:], lh[:], e[:], op=Alu.mult)
            Z = hot.tile([128, NB * H], F32, tag="Z")
            S = hot.tile([128, NB * H], F32, tag="S")
            for r in range(NB):
                nc.vector.tensor_reduce(
                    Z[:, r * H:(r + 1) * H],
                    _v(e, r * C * H, [(1, H), (H, C)]), axis=AX, op=Alu.add)
                nc.vector.tensor_reduce(
                    S[:, r * H:(r + 1) * H],
                    _v(el, r * C * H, [(1, H), (H, C)]), axis=AX, op=Alu.add)
            rZ = hot.tile([128, NB * H], F32, tag="rZ")
            nc.vector.reciprocal(rZ[:], Z[:])
            lnZ = hot.tile([128, NB * H], F32, tag="lnZ")
            nc.scalar.activation(lnZ[:], Z[:], Act.Ln)
            t1 = hot.tile([128, NB * H], F32, tag="t1")
            nc.vector.tensor_tensor(t1[:], S[:], rZ[:], op=Alu.mult)
            Ht = hot.tile([128, NB * H], F32, tag="Ht")
            nc.vector.tensor_tensor(Ht[:], lnZ[:], t1[:], op=Alu.subtract)

            # ---- phase 2: dedup mask + cumsum ----
            mask = hot.tile([128, NB * H], BF16, tag="mask")
            nc.vector.tensor_tensor(
                _v(mask, 1, [(H, NB), (1, H - 1)]),
                _v(predsh, 1, [(H, NB), (1, H - 1)]),
                _v(predsh, 0, [(H, NB), (1, H - 1)]), op=Alu.not_equal)
            # cols >= 1: mask &= (pred != blank);  col 0: mask = (pred != blank)
            nc.vector.scalar_tensor_tensor(
                _v(mask, 1, [(H, NB), (1, H - 1)]),
                _v(predsh, 1, [(H, NB), (1, H - 1)]), 0.0,
                _v(mask, 1, [(H, NB), (1, H - 1)]),
                op0=Alu.not_equal, op1=Alu.logical_and)
            nc.vector.tensor_scalar(
                _v(mask, 0, [(H, NB), (1, 1)]),
                _v(predsh, 0, [(H, NB), (1, 1)]), 0.0, None,
                op0=Alu.not_equal)
            pos1 = hot.tile([128, NB * H], F32, tag="pos1")
            for r in range(NB):
                sl = slice(r * H, (r + 1) * H)
                nc.vector.tensor_tensor_scan(
                    pos1[:, sl], mask[:, sl], mask[:, sl], 0.0,
                    op0=Alu.add, op1=Alu.max)

            # ---- phase 3b: one-hot slot extraction ----
            # ind layout (r, j, t): off = (r*JW + j)*H + t
            ind = hot.tile([128, NB * JW * H], F32, tag="ind")
            dtmp = hot.tile([128, NB * JW * H], F32, tag="dtmp")
            ctmp = hot.tile([128, NB * JW * H], F32, tag="ctmp")
            for r in range(NB):
                o = r * JW * H
                iv = _v(ind, o, [(H, JW), (1, H)])
                nc.vector.tensor_tensor(
                    iv, _v(pos1, r * H, [(0, JW), (1, H)]),
                    _v(jio, 0, [(1, JW), (0, H)]), op=Alu.is_equal)
                nc.vector.tensor_tensor(
                    iv, iv, _v(mask, r * H, [(0, JW), (1, H)]),
                    op=Alu.logical_and)
                nc.vector.tensor_tensor(
                    _v(dtmp, o, [(H, JW), (1, H)]), iv,
                    _v(predsh, r * H, [(0, JW), (1, H)]), op=Alu.mult)
                nc.vector.tensor_tensor(
                    _v(ctmp, o, [(H, JW), (1, H)]), iv,
                    _v(Ht, r * H, [(0, JW), (1, H)]), op=Alu.mult)

            dec_acc = hot.tile([128, NB * JW], F32, tag="dec_acc")
            cnt_acc = hot.tile([128, NB * JW], F32, tag="cnt_acc")
            cf_acc = hot.tile([128, NB * JW], F32, tag="cf_acc")
            rdim = [(H, NB * JW), (1, H)]
            nc.vector.tensor_reduce(dec_acc[:], _v(dtmp, 0, rdim), axis=AX,
                                    op=Alu.add)
            nc.vector.tensor_reduce(cnt_acc[:], _v(ind, 0, rdim), axis=AX,
                                    op=Alu.add)
            nc.vector.tensor_reduce(cf_acc[:], _v(ctmp, 0, rdim), axis=AX,
                                    op=Alu.add)

            # ---- flag: any row with pos1[H-1] < 6 needs the cold path ----
            rflag2 = hot.tile([128, NB], F32, tag="rflag2")
            nc.vector.tensor_scalar(rflag2[:], _v(pos1, H - 1, [(H, NB), (1, 1)]),
                                    float(MAXLEN), None, op0=Alu.is_lt)
            rflagr = hot.tile([128, 1], F32, tag="rflagr")
            nc.vector.tensor_reduce(rflagr[:], rflag2[:], axis=AX, op=Alu.add)
            fl_ps = psum_pool.tile([1, 1], F32, tag="fl_ps")
            nc.tensor.matmul(fl_ps[:], rflagr[:], ones[:], start=True, stop=True)
            fl_sb = hot.tile([1, 1], I32, tag="fl_sb")
            nc.vector.tensor_copy(fl_sb[:], fl_ps[:])
            fv = nc.values_load(fl_sb[:], min_val=0, max_val=257,
                                skip_runtime_bounds_check=True)

            # ================= COLD PATH (worst-case guard) =================
            # Statistically never taken: full-T recompute of preds/mask/pos1,
            # then accumulate slot contributions from t >= H into the accs.
            with tc.If(fv >= 1):
                TcC = 256
                predsC_b, maskC_b, pos1C_b = [], [], []
                for bc in range(NB):
                    b0 = bc * 128
                    predsC = cperbc.tile([128, T], BF16, tag="predsC")
                    for k in range(T // TcC):
                        t0 = k * TcC
                        lt = clt.tile([128, C * TcC], F32, tag="lt")
                        lt_ct = _v(lt, 0, [(TcC, C), (1, TcC)])
                        lt_tc = _v(lt, 0, [(1, TcC), (TcC, C)])
                        nc.sync.dma_start(lt_ct, lg[b0:b0 + 128, :, t0:t0 + TcC])
                        mC = cm.tile([128, TcC], F32, tag="mC")
                        nc.vector.tensor_reduce(mC[:], lt_tc, axis=AX, op=Alu.max)
                        eqC = ceq.tile([128, C * TcC], BF16, tag="eqC")
                        eq_tc = _v(eqC, 0, [(C, TcC), (1, C)])
                        m_bc = _v(mC, 0, [(1, TcC), (0, C)])
                        nc.vector.scalar_tensor_tensor(
                            eq_tc, m_bc, 1.0, lt_tc, op0=Alu.mult, op1=Alu.is_le)
                        wC = ceq.tile([128, C * TcC], BF16, tag="wC")
                        w_tc = _v(wC, 0, [(C, TcC), (1, C)])
                        cio_bc = _v(cio, 0, [(0, TcC), (1, C)])
                        nc.vector.tensor_tensor(w_tc, eq_tc, cio_bc, op=Alu.mult)
                        nc.vector.tensor_reduce(predsC[:, t0:t0 + TcC], w_tc,
                                                axis=AX, op=Alu.max)
                    maskC = cperbc.tile([128, T], BF16, tag="maskC")
                    nc.vector.memset(maskC[:, 0:1], 1.0)
                    nc.vector.tensor_tensor(maskC[:, 1:T], predsC[:, 1:T],
                                            predsC[:, 0:T - 1], op=Alu.not_equal)
                    nc.vector.scalar_tensor_tensor(
                        maskC[:], predsC[:], 0.0, maskC[:],
                        op0=Alu.not_equal, op1=Alu.logical_and)
                    pos1C = cperbc.tile([128, T], F32, tag="pos1C")
                    nc.vector.tensor_tensor_scan(
                        pos1C[:], maskC[:], maskC[:], 0.0,
                        op0=Alu.add, op1=Alu.max)
                    predsC_b.append(predsC)
                    maskC_b.append(maskC)
                    pos1C_b.append(pos1C)

                for bc in range(NB):
                    b0 = bc * 128
                    asl = slice(bc * JW, (bc + 1) * JW)
                    for Sc in range(H, T, 128):
                        Ec = min(Sc + 128, T)
                        sz = Ec - Sc
                        lh3 = cph3.tile([128, C * sz], F32, tag="lh3")
                        nc.gpsimd.dma_start(_v(lh3, 0, [(sz, C), (1, sz)]),
                                            lg[b0:b0 + 128, :, Sc:Ec])
                        e3 = cph3.tile([128, C * sz], F32, tag="e3")
                        nc.scalar.activation(e3[:], lh3[:], Act.Exp)
                        el3 = cph3.tile([128, C * sz], F32, tag="el3")
                        nc.vector.tensor_tensor(el3[:], lh3[:], e3[:], op=Alu.mult)
                        Z3 = cph3.tile([128, sz], F32, tag="Z3")
                        nc.vector.tensor_reduce(Z3[:], _v(e3, 0, [(1, sz), (sz, C)]),
                                                axis=AX, op=Alu.add)
                        S3 = cph3.tile([128, sz], F32, tag="S3")
                        nc.vector.tensor_reduce(S3[:], _v(el3, 0, [(1, sz), (sz, C)]),
                                                axis=AX, op=Alu.add)
                        rZ3 = cph3.tile([128, sz], F32, tag="rZ3")
                        nc.vector.reciprocal(rZ3[:], Z3[:])
                        lnZ3 = cph3.tile([128, sz], F32, tag="lnZ3")
                        nc.scalar.activation(lnZ3[:], Z3[:], Act.Ln)
                        t13 = cph3.tile([128, sz], F32, tag="t13")
                        nc.vector.tensor_tensor(t13[:], S3[:], rZ3[:], op=Alu.mult)
                        Ht3 = cph3.tile([128, sz], F32, tag="Ht3")
                        nc.vector.tensor_tensor(Ht3[:], lnZ3[:], t13[:],
                                                op=Alu.subtract)

                        pos1C, maskC, predsC = pos1C_b[bc], maskC_b[bc], predsC_b[bc]
                        p1s = _v(pos1C, Sc, [(0, JW), (1, sz)])
                        msks = _v(maskC, Sc, [(0, JW), (1, sz)])
                        prds = _v(predsC, Sc, [(0, JW), (1, sz)])
                        jio_bc2 = _v(jio, 0, [(1, JW), (0, sz)])
                        ind3 = cph3.tile([128, JW * sz], F32, tag="ind3")
                        ind3_v = _v(ind3, 0, [(sz, JW), (1, sz)])
                        nc.vector.tensor_tensor(ind3_v, p1s, jio_bc2,
                                                op=Alu.is_equal)
                        nc.vector.tensor_tensor(ind3_v, ind3_v, msks,
                                                op=Alu.logical_and)

                        tmp3 = cph3.tile([128, JW * sz], F32, tag="tmp3")
                        tmp3_v = _v(tmp3, 0, [(sz, JW), (1, sz)])
                        red = cph3.tile([128, JW], F32, tag="red")
                        nc.vector.tensor_tensor(tmp3_v, ind3_v, prds, op=Alu.mult)
                        nc.vector.tensor_reduce(red[:], tmp3_v, axis=AX, op=Alu.add)
                        nc.vector.tensor_tensor(dec_acc[:, asl], dec_acc[:, asl],
                                                red[:], op=Alu.add)
                        red2 = cph3.tile([128, JW], F32, tag="red2")
                        nc.vector.tensor_reduce(red2[:], ind3_v, axis=AX, op=Alu.add)
                        nc.vector.tensor_tensor(cnt_acc[:, asl], cnt_acc[:, asl],
                                                red2[:], op=Alu.add)
                        Ht3_bv = _v(Ht3, 0, [(0, JW), (1, sz)])
                        nc.vector.tensor_tensor(tmp3_v, ind3_v, Ht3_bv, op=Alu.mult)
                        red3 = cph3.tile([128, JW], F32, tag="red3")
                        nc.vector.tensor_reduce(red3[:], tmp3_v, axis=AX, op=Alu.add)
                        nc.vector.tensor_tensor(cf_acc[:, asl], cf_acc[:, asl],
                                                red3[:], op=Alu.add)

            # ==================== finalize + output ====================
            # dec_acc holds sum(ind * (11-pred)) = cnt*11 - pred_true.
            # dec = 12*cnt - dec_acc - 1   (cnt in {0,1}; empty -> -1)
            decf = hot.tile([128, NB * JW], F32, tag="decf")
            nc.vector.scalar_tensor_tensor(decf[:], cnt_acc[:], 12.0, dec_acc[:],
                                           op0=Alu.mult, op1=Alu.subtract)
            nc.vector.tensor_scalar_sub(decf[:], decf[:], 1.0)
            deci = hot.tile([128, NB * JW], I32, tag="deci")
            nc.vector.tensor_copy(deci[:], decf[:])

            nc.sync.dma_start(dec_o[0:128, :], deci[:, 0:JW])
            nc.scalar.dma_start(dec_o[128:256, :], deci[:, JW:2 * JW])
            nc.gpsimd.dma_start(conf_o[0:128, :], cf_acc[:, 0:JW])
            nc.sync.dma_start(conf_o[128:256, :], cf_acc[:, JW:2 * JW])

    return nc


_CACHED = {}


def _get_program(B, T, head=HEAD):
    key = (B, T, head)
    if key not in _CACHED:
        nc = bacc.Bacc()
        build_decoder(nc, B, T, head=head)
        nc.compile()
        _CACHED[key] = nc
    return _CACHED[key]


def kernel(logits: np.ndarray):
    logits = np.ascontiguousarray(logits, dtype=np.float32)
    B, c, T = logits.shape
    assert c == C
    Bs = B // N_CORES
    nc = _get_program(Bs, T)
    in_maps = [
        {"logits": logits[i * Bs:(i + 1) * Bs]} for i in range(N_CORES)
    ]
    res = run_bass_kernel_spmd(nc, in_maps, core_ids=list(range(N_CORES)))
    dec = np.concatenate([r["decoded"] for r in res.results], axis=0)
    conf = np.concatenate([r["confidences"] for r in res.results], axis=0)
    return dec.astype(np.int32), conf.astype(np.float32)


# revision 10
# speedup vs baseline: 1.2177x; 1.2177x over previous
"""CRNN greedy CTC-style decoder kernel for Trainium2 (Bass/Tile).

Problem: logits [B=2048, C=12, T=2048] f32 ->
  decoded     [B, 6] int32  (first 6 CTC-collapsed tokens, pad -1)
  confidences [B, 6] f32    (per-kept-timestep softmax entropy, pad 0)

Sharding: pure data-parallel over batch across 8 NeuronCores
(256 rows/core), no communication.

Key observation: with i.i.d. logits the keep probability per timestep is
(11/12)^2 ~ 0.84, so every row resolves its 6 output slots within the
first few timesteps (measured max t = 11 over the full input).  The hot
path therefore only reads/decodes logits[:, :, 0:HEAD] (HEAD=12):

  Hot path (always runs, 2 row-halves packed per partition):
    phase 1: exact argmax over C via max / one-hot(is_le) / max-of
      eq*(11-c) chain -- bit-exact ties vs jnp.argmax (smallest index).
    phase 2: run-dedup mask, inclusive cumsum (scan) -> pos1.
    phase 3: entropy H = lnZ - (sum_c e^l * l)/Z (exact identity; the
      reference's +1e-6 inside the log shifts H by only ~1e-5 relative;
      no max-subtraction needed since |l| <= ~6 for randn inputs), slot
      extraction via one-hot ind = (pos1==j+1 & mask):
        u  = sum_t ind*((11-pred)+16)  -> decoded = 28*(u>0) - 1 - u
        cf = sum_t ind*H               -> confidences
    All elementwise/reduce work on DVE (HW Pool engine lacks these
    opcodes); exp/ln on Act engine; iota/one DMA queue on Pool.

  Flag: one PE matmul counts rows with pos1[HEAD-1] < 6.  If any row is
  unresolved (statistically never; impossible for the seed-0 input), a
  guarded cold path recomputes preds/mask/pos1 over the full T and
  accumulates slot contributions from t >= HEAD, preserving worst-case
  correctness for arbitrary inputs.

Perf: 211934 ns baseline (full-T argmax sweep, DVE-bound) -> head-gated
hot path is input-DMA latency + ~5 us DVE chain + output-DMA latency.
"""

import numpy as np

import concourse.bass as bass
import concourse.bacc as bacc
import concourse.mybir as mybir
import concourse.tile as tile
from concourse.bass_utils import run_bass_kernel_spmd

F32 = mybir.dt.float32
BF16 = mybir.dt.bfloat16
I32 = mybir.dt.int32
Alu = mybir.AluOpType
Act = mybir.ActivationFunctionType
AX = mybir.AxisListType.X

N_CORES = 8
MAXLEN = 6
BLANK = 11
PAD = -1

# full problem shape (hardcoded per the harness contract)
B_FULL, C, T_FULL = 2048, 12, 2048
JW = MAXLEN
HEAD = 12


def _v(t, off, dims):
    """AP on tile t at element offset `off`: dims = [(step, count), ...]."""
    ap = t[:]
    return bass.AP(ap.tensor, ap.offset + off, [ap.ap[0]] + [list(d) for d in dims])


def build_decoder(nc, B, T, head=HEAD):
    """Emit the per-core decoder program.  B = rows per core (must be 256)."""
    assert B == 256, "hot path packs exactly 2 row-halves per partition"
    H = head
    NB = B // 128  # = 2 row-halves

    lg = nc.dram_tensor("logits", [B, C, T], F32, kind="ExternalInput")
    dec_o = nc.dram_tensor("decoded", [B, MAXLEN], I32, kind="ExternalOutput")
    conf_o = nc.dram_tensor("confidences", [B, MAXLEN], F32, kind="ExternalOutput")

    with tile.TileContext(nc) as tc:
        with (
            tc.tile_pool(name="consts", bufs=1) as consts,
            tc.tile_pool(name="hot", bufs=1) as hot,
            tc.tile_pool(name="clt", bufs=2) as clt,
            tc.tile_pool(name="ceq", bufs=2) as ceq,
            tc.tile_pool(name="cm", bufs=2) as cm,
            tc.tile_pool(name="cperbc", bufs=NB) as cperbc,
            tc.tile_pool(name="cph3", bufs=2) as cph3,
            tc.tile_pool(name="psum", bufs=1, space="PSUM") as psum_pool,
        ):
            # ---------------- constants ----------------
            # reversed class weights 11-c: argmax extracted via MAX of
            # eq*(11-c) -> smallest class index wins ties (= jnp.argmax).
            cio_i = consts.tile([128, C], I32, tag="cio_i")
            nc.gpsimd.iota(cio_i[:], pattern=[[-1, C]], base=C - 1,
                           channel_multiplier=0)
            cio = consts.tile([128, C], BF16, tag="cio")
            nc.vector.tensor_copy(cio[:], cio_i[:])

            jio_i = consts.tile([128, JW], I32, tag="jio_i")
            nc.gpsimd.iota(jio_i[:], pattern=[[1, JW]], base=1,
                           channel_multiplier=0)
            jio = consts.tile([128, JW], F32, tag="jio")
            nc.vector.tensor_copy(jio[:], jio_i[:])

            ones = consts.tile([128, 1], F32, tag="ones")
            nc.vector.memset(ones[:], 1.0)

            # ================= HOT PATH =================
            # lh layout (r, c, t): off = (r*C + c)*H + t
            lh = hot.tile([128, NB * C * H], F32, tag="lh")
            for r in range(NB):
                dst = _v(lh, r * C * H, [(H, C), (1, H)])
                src = lg[r * 128:(r + 1) * 128, :, 0:H]
                (nc.sync if r == 0 else nc.scalar).dma_start(dst, src)

            # ---- phase 1: exact argmax (DVE) ----
            m = hot.tile([128, NB * H], F32, tag="m")
            nc.vector.tensor_reduce(
                m[:], _v(lh, 0, [(C * H, NB), (1, H), (H, C)]),
                axis=AX, op=Alu.max)
            # eq layout (r, t, c): off = (r*H + t)*C + c  (c contiguous)
            # (TensorScalarPtr is capped at 2 free dims -> per-row-half)
            eq = hot.tile([128, NB * H * C], BF16, tag="eq")
            for r in range(NB):
                nc.vector.scalar_tensor_tensor(
                    _v(eq, r * H * C, [(C, H), (1, C)]),
                    _v(m, r * H, [(1, H), (0, C)]), 1.0,
                    _v(lh, r * C * H, [(1, H), (H, C)]),
                    op0=Alu.mult, op1=Alu.is_le)
            w = hot.tile([128, NB * H * C], BF16, tag="w")
            eq_v = _v(eq, 0, [(C, NB * H), (1, C)])
            w_v = _v(w, 0, [(C, NB * H), (1, C)])
            nc.vector.tensor_tensor(w_v, eq_v,
                                    _v(cio, 0, [(0, NB * H), (1, C)]),
                                    op=Alu.mult)
            # predsh[:, r*H + t] = 11 - argmax_c  (blank=11 -> 0)
            predsh = hot.tile([128, NB * H], BF16, tag="predsh")
            nc.vector.tensor_reduce(predsh[:], w_v, axis=AX, op=Alu.max)

            # ---- phase 2: dedup mask + cumsum (DVE) ----
            mask = hot.tile([128, NB * H], BF16, tag="mask")
            nc.vector.tensor_tensor(
                _v(mask, 1, [(H, NB), (1, H - 1)]),
                _v(predsh, 1, [(H, NB), (1, H - 1)]),
                _v(predsh, 0, [(H, NB), (1, H - 1)]), op=Alu.not_equal)
            # cols >= 1: mask &= (pred != blank);  col 0: mask = (pred != blank)
            nc.vector.scalar_tensor_tensor(
                _v(mask, 1, [(H, NB), (1, H - 1)]),
                _v(predsh, 1, [(H, NB), (1, H - 1)]), 0.0,
                _v(mask, 1, [(H, NB), (1, H - 1)]),
                op0=Alu.not_equal, op1=Alu.logical_and)
            nc.vector.tensor_scalar(
                _v(mask, 0, [(H, NB), (1, 1)]),
                _v(predsh, 0, [(H, NB), (1, 1)]), 0.0, None,
                op0=Alu.not_equal)
            pos1 = hot.tile([128, NB * H], F32, tag="pos1")
            for r in range(NB):
                sl = slice(r * H, (r + 1) * H)
                nc.vector.tensor_tensor_scan(
                    pos1[:, sl], mask[:, sl], mask[:, sl], 0.0,
                    op0=Alu.add, op1=Alu.max)

            # ---- flag (early, so idle engines can branch + prefire DMAs):
            # any row with pos1[H-1] < 6 needs the cold path
            rflag2 = hot.tile([128, NB], F32, tag="rflag2")
            nc.vector.tensor_scalar(rflag2[:],
                                    _v(pos1, H - 1, [(H, NB), (1, 1)]),
                                    float(MAXLEN), None, op0=Alu.is_lt)
            rflagr = hot.tile([128, 1], F32, tag="rflagr")
            nc.vector.tensor_reduce(rflagr[:], rflag2[:], axis=AX, op=Alu.add)
            fl_ps = psum_pool.tile([1, 1], F32, tag="fl_ps")
            nc.tensor.matmul(fl_ps[:], rflagr[:], ones[:], start=True, stop=True)
            fl_sb = hot.tile([1, 1], I32, tag="fl_sb")
            nc.vector.tensor_copy(fl_sb[:], fl_ps[:])
            fv = nc.values_load(fl_sb[:], min_val=0, max_val=257,
                                skip_runtime_bounds_check=True)

            # ---- phase 3a: entropy (Act: exp/ln; DVE: el, Z|S, H) ----
            # ee = [e | el], e = exp(l)  (no max-subtract: |l| <= ~6 is safe)
            ee = hot.tile([128, 2 * NB * C * H], F32, tag="ee")
            nc.scalar.activation(_v(ee, 0, [(1, NB * C * H)]), lh[:], Act.Exp)
            nc.vector.tensor_tensor(_v(ee, NB * C * H, [(1, NB * C * H)]),
                                    lh[:], _v(ee, 0, [(1, NB * C * H)]),
                                    op=Alu.mult)
            # ZS = [Z | S]: one fused reduce over c for both halves
            ZS = hot.tile([128, 2 * NB * H], F32, tag="ZS")
            nc.vector.tensor_reduce(
                ZS[:], _v(ee, 0, [(C * H, 2 * NB), (1, H), (H, C)]),
                axis=AX, op=Alu.add)
            Zv = _v(ZS, 0, [(1, NB * H)])
            Sv = _v(ZS, NB * H, [(1, NB * H)])
            lnZ = hot.tile([128, NB * H], F32, tag="lnZ")
            nc.scalar.activation(lnZ[:], Zv, Act.Ln)
            t1 = hot.tile([128, NB * H], F32, tag="t1")
            nc.vector.tensor_tensor(t1[:], Sv, Zv, op=Alu.divide)
            Ht = hot.tile([128, NB * H], F32, tag="Ht")
            nc.vector.tensor_tensor(Ht[:], lnZ[:], t1[:], op=Alu.subtract)

            # ---- phase 3b: one-hot slot extraction ----
            # ind layout (r, j, t): off = (r*JW + j)*H + t
            ind = hot.tile([128, NB * JW * H], F32, tag="ind")
            utmp = hot.tile([128, NB * JW * H], F32, tag="utmp")
            ctmp = hot.tile([128, NB * JW * H], F32, tag="ctmp")
            for r in range(NB):
                o = r * JW * H
                iv = _v(ind, o, [(H, JW), (1, H)])
                nc.vector.tensor_tensor(
                    iv, _v(pos1, r * H, [(0, JW), (1, H)]),
                    _v(jio, 0, [(1, JW), (0, H)]), op=Alu.is_equal)
                nc.vector.tensor_tensor(
                    iv, iv, _v(mask, r * H, [(0, JW), (1, H)]),
                    op=Alu.logical_and)
                # utmp = ((11-pred) + 16) * ind
                nc.vector.scalar_tensor_tensor(
                    _v(utmp, o, [(H, JW), (1, H)]),
                    _v(predsh, r * H, [(0, JW), (1, H)]), 16.0, iv,
                    op0=Alu.add, op1=Alu.mult)
                nc.vector.tensor_tensor(
                    _v(ctmp, o, [(H, JW), (1, H)]), iv,
                    _v(Ht, r * H, [(0, JW), (1, H)]), op=Alu.mult)

            u_acc = hot.tile([128, NB * JW], F32, tag="u_acc")
            cf_acc = hot.tile([128, NB * JW], F32, tag="cf_acc")
            rdim = [(H, NB * JW), (1, H)]
            nc.vector.tensor_reduce(u_acc[:], _v(utmp, 0, rdim), axis=AX,
                                    op=Alu.add)
            nc.vector.tensor_reduce(cf_acc[:], _v(ctmp, 0, rdim), axis=AX,
                                    op=Alu.add)

            # ================= COLD PATH (worst-case guard) =================
            # Statistically never taken: full-T recompute of preds/mask/pos1,
            # then accumulate slot contributions from t >= H into the accs.
            with tc.If(fv >= 1):
                TcC = 256
                predsC_b, maskC_b, pos1C_b = [], [], []
                for bc in range(NB):
                    b0 = bc * 128
                    predsC = cperbc.tile([128, T], BF16, tag="predsC")
                    for k in range(T // TcC):
                        t0 = k * TcC
                        lt = clt.tile([128, C * TcC], F32, tag="lt")
                        lt_ct = _v(lt, 0, [(TcC, C), (1, TcC)])
                        lt_tc = _v(lt, 0, [(1, TcC), (TcC, C)])
                        nc.sync.dma_start(lt_ct, lg[b0:b0 + 128, :, t0:t0 + TcC])
                        mC = cm.tile([128, TcC], F32, tag="mC")
                        nc.vector.tensor_reduce(mC[:], lt_tc, axis=AX, op=Alu.max)
                        eqC = ceq.tile([128, C * TcC], BF16, tag="eqC")
                        eq_tc = _v(eqC, 0, [(C, TcC), (1, C)])
                        m_bc = _v(mC, 0, [(1, TcC), (0, C)])
                        nc.vector.scalar_tensor_tensor(
                            eq_tc, m_bc, 1.0, lt_tc, op0=Alu.mult, op1=Alu.is_le)
                        wC = ceq.tile([128, C * TcC], BF16, tag="wC")
                        w_tc = _v(wC, 0, [(C, TcC), (1, C)])
                        cio_bc = _v(cio, 0, [(0, TcC), (1, C)])
                        nc.vector.tensor_tensor(w_tc, eq_tc, cio_bc, op=Alu.mult)
                        nc.vector.tensor_reduce(predsC[:, t0:t0 + TcC], w_tc,
                                                axis=AX, op=Alu.max)
                    maskC = cperbc.tile([128, T], BF16, tag="maskC")
                    nc.vector.memset(maskC[:, 0:1], 1.0)
                    nc.vector.tensor_tensor(maskC[:, 1:T], predsC[:, 1:T],
                                            predsC[:, 0:T - 1], op=Alu.not_equal)
                    nc.vector.scalar_tensor_tensor(
                        maskC[:], predsC[:], 0.0, maskC[:],
                        op0=Alu.not_equal, op1=Alu.logical_and)
                    pos1C = cperbc.tile([128, T], F32, tag="pos1C")
                    nc.vector.tensor_tensor_scan(
                        pos1C[:], maskC[:], maskC[:], 0.0,
                        op0=Alu.add, op1=Alu.max)
                    predsC_b.append(predsC)
                    maskC_b.append(maskC)
                    pos1C_b.append(pos1C)

                for bc in range(NB):
                    b0 = bc * 128
                    asl = slice(bc * JW, (bc + 1) * JW)
                    for Sc in range(H, T, 128):
                        Ec = min(Sc + 128, T)
                        sz = Ec - Sc
                        lh3 = cph3.tile([128, C * sz], F32, tag="lh3")
                        nc.gpsimd.dma_start(_v(lh3, 0, [(sz, C), (1, sz)]),
                                            lg[b0:b0 + 128, :, Sc:Ec])
                        e3 = cph3.tile([128, C * sz], F32, tag="e3")
                        nc.scalar.activation(e3[:], lh3[:], Act.Exp)
                        el3 = cph3.tile([128, C * sz], F32, tag="el3")
                        nc.vector.tensor_tensor(el3[:], lh3[:], e3[:], op=Alu.mult)
                        Z3 = cph3.tile([128, sz], F32, tag="Z3")
                        nc.vector.tensor_reduce(Z3[:], _v(e3, 0, [(1, sz), (sz, C)]),
                                                axis=AX, op=Alu.add)
                        S3 = cph3.tile([128, sz], F32, tag="S3")
                        nc.vector.tensor_reduce(S3[:], _v(el3, 0, [(1, sz), (sz, C)]),
                                                axis=AX, op=Alu.add)
                        lnZ3 = cph3.tile([128, sz], F32, tag="lnZ3")
                        nc.scalar.activation(lnZ3[:], Z3[:], Act.Ln)
                        t13 = cph3.tile([128, sz], F32, tag="t13")
                        nc.vector.tensor_tensor(t13[:], S3[:], Z3[:], op=Alu.divide)
                        Ht3 = cph3.tile([128, sz], F32, tag="Ht3")
                        nc.vector.tensor_tensor(Ht3[:], lnZ3[:], t13[:],
                                                op=Alu.subtract)

                        pos1C, maskC, predsC = pos1C_b[bc], maskC_b[bc], predsC_b[bc]
                        p1s = _v(pos1C, Sc, [(0, JW), (1, sz)])
                        msks = _v(maskC, Sc, [(0, JW), (1, sz)])
                        prds = _v(predsC, Sc, [(0, JW), (1, sz)])
                        jio_bc2 = _v(jio, 0, [(1, JW), (0, sz)])
                        ind3 = cph3.tile([128, JW * sz], F32, tag="ind3")
                        ind3_v = _v(ind3, 0, [(sz, JW), (1, sz)])
                        nc.vector.tensor_tensor(ind3_v, p1s, jio_bc2,
                                                op=Alu.is_equal)
                        nc.vector.tensor_tensor(ind3_v, ind3_v, msks,
                                                op=Alu.logical_and)

                        tmp3 = cph3.tile([128, JW * sz], F32, tag="tmp3")
                        tmp3_v = _v(tmp3, 0, [(sz, JW), (1, sz)])
                        red = cph3.tile([128, JW], F32, tag="red")
                        nc.vector.scalar_tensor_tensor(
                            tmp3_v, prds, 16.0, ind3_v,
                            op0=Alu.add, op1=Alu.mult)
                        nc.vector.tensor_reduce(red[:], tmp3_v, axis=AX, op=Alu.add)
                        nc.vector.tensor_tensor(u_acc[:, asl], u_acc[:, asl],
                                                red[:], op=Alu.add)
                        Ht3_bv = _v(Ht3, 0, [(0, JW), (1, sz)])
                        nc.vector.tensor_tensor(tmp3_v, ind3_v, Ht3_bv, op=Alu.mult)
                        red3 = cph3.tile([128, JW], F32, tag="red3")
                        nc.vector.tensor_reduce(red3[:], tmp3_v, axis=AX, op=Alu.add)
                        nc.vector.tensor_tensor(cf_acc[:, asl], cf_acc[:, asl],
                                                red3[:], op=Alu.add)

            # ==================== finalize + output ====================
            # u = (11-pred) + 16 for a filled slot, 0 for empty.
            # dec = (28*(u>0) - 1) - u   (filled -> pred; empty -> -1)
            decf = hot.tile([128, NB * JW], F32, tag="decf")
            nc.vector.tensor_scalar(decf[:], u_acc[:], 0.0, 28.0,
                                    op0=Alu.is_gt, op1=Alu.mult)
            nc.vector.scalar_tensor_tensor(decf[:], decf[:], -1.0, u_acc[:],
                                           op0=Alu.add, op1=Alu.subtract)
            deci = hot.tile([128, NB * JW], I32, tag="deci")
            nc.vector.tensor_copy(deci[:], decf[:])

            # one DMA per output tensor: SBUF (r,j) -> DRAM both row-halves
            dap = dec_o[0:128, :]
            nc.sync.dma_start(
                bass.AP(dap.tensor, dap.offset,
                        [dap.ap[0], [128 * JW, NB], [1, JW]]), deci[:])
            cap = conf_o[0:128, :]
            nc.scalar.dma_start(
                bass.AP(cap.tensor, cap.offset,
                        [cap.ap[0], [128 * JW, NB], [1, JW]]), cf_acc[:])

    return nc


_CACHED = {}


def _get_program(B, T, head=HEAD):
    key = (B, T, head)
    if key not in _CACHED:
        nc = bacc.Bacc()
        build_decoder(nc, B, T, head=head)
        nc.compile()
        _CACHED[key] = nc
    return _CACHED[key]


def kernel(logits: np.ndarray):
    logits = np.ascontiguousarray(logits, dtype=np.float32)
    B, c, T = logits.shape
    assert c == C
    Bs = B // N_CORES
    nc = _get_program(Bs, T)
    in_maps = [
        {"logits": logits[i * Bs:(i + 1) * Bs]} for i in range(N_CORES)
    ]
    res = run_bass_kernel_spmd(nc, in_maps, core_ids=list(range(N_CORES)))
    dec = np.concatenate([r["decoded"] for r in res.results], axis=0)
    conf = np.concatenate([r["confidences"] for r in res.results], axis=0)
    return dec.astype(np.int32), conf.astype(np.float32)


# revision 13
# speedup vs baseline: 1.2483x; 1.0251x over previous
"""CRNN greedy CTC-style decoder kernel for Trainium2 (Bass/Tile).

Problem: logits [B=2048, C=12, T=2048] f32 ->
  decoded     [B, 6] int32  (first 6 CTC-collapsed tokens, pad -1)
  confidences [B, 6] f32    (per-kept-timestep softmax entropy, pad 0)

Sharding: pure data-parallel over batch across 8 NeuronCores
(256 rows/core), no communication.

Key observation: with i.i.d. logits the keep probability per timestep is
(11/12)^2 ~ 0.84, so every row resolves its 6 output slots within the
first few timesteps (measured max t = 11 over the full input).  The hot
path therefore only reads/decodes logits[:, :, 0:HEAD] (HEAD=12):

  Hot path (always runs, 2 row-halves packed per partition):
    phase 1: exact argmax over C via max / one-hot(is_le) / max-of
      eq*(11-c) chain -- bit-exact ties vs jnp.argmax (smallest index).
    phase 2: run-dedup mask, inclusive cumsum (scan) -> pos1.
    phase 3: entropy H = lnZ - (sum_c e^l * l)/Z (exact identity; the
      reference's +1e-6 inside the log shifts H by only ~1e-5 relative;
      no max-subtraction needed since |l| <= ~6 for randn inputs), slot
      extraction via one-hot ind = (pos1==j+1 & mask):
        u  = sum_t ind*((11-pred)+16)  -> decoded = 28*(u>0) - 1 - u
        cf = sum_t ind*H               -> confidences
    All elementwise/reduce work on DVE (HW Pool engine lacks these
    opcodes); exp/ln on Act engine; iota/one DMA queue on Pool.

  Flag: one PE matmul counts rows with pos1[HEAD-1] < 6.  If any row is
  unresolved (statistically never; impossible for the seed-0 input), a
  guarded cold path recomputes preds/mask/pos1 over the full T and
  accumulates slot contributions from t >= HEAD, preserving worst-case
  correctness for arbitrary inputs.

Perf: 211934 ns baseline (full-T argmax sweep, DVE-bound) -> head-gated
hot path is input-DMA latency + ~5 us DVE chain + output-DMA latency.
"""

import numpy as np

import concourse.bass as bass
import concourse.bacc as bacc
import concourse.mybir as mybir
import concourse.tile as tile
from concourse.bass_utils import run_bass_kernel_spmd

F32 = mybir.dt.float32
BF16 = mybir.dt.bfloat16
I32 = mybir.dt.int32
Alu = mybir.AluOpType
Act = mybir.ActivationFunctionType
AX = mybir.AxisListType.X

N_CORES = 8
MAXLEN = 6
BLANK = 11
PAD = -1

# full problem shape (hardcoded per the harness contract)
B_FULL, C, T_FULL = 2048, 12, 2048
JW = MAXLEN
HEAD = 12


def _v(t, off, dims):
    """AP on tile t at element offset `off`: dims = [(step, count), ...]."""
    ap = t[:]
    return bass.AP(ap.tensor, ap.offset + off, [ap.ap[0]] + [list(d) for d in dims])


def build_decoder(nc, B, T, head=HEAD):
    """Emit the per-core decoder program.  B = rows per core (must be 256)."""
    assert B == 256, "hot path packs exactly 2 row-halves per partition"
    H = head
    NB = B // 128  # = 2 row-halves

    lg = nc.dram_tensor("logits", [B, C, T], F32, kind="ExternalInput")
    dec_o = nc.dram_tensor("decoded", [B, MAXLEN], I32, kind="ExternalOutput")
    conf_o = nc.dram_tensor("confidences", [B, MAXLEN], F32, kind="ExternalOutput")

    with tile.TileContext(nc) as tc:
        with (
            tc.tile_pool(name="consts", bufs=1) as consts,
            tc.tile_pool(name="hot", bufs=1) as hot,
            tc.tile_pool(name="clt", bufs=2) as clt,
            tc.tile_pool(name="ceq", bufs=2) as ceq,
            tc.tile_pool(name="cm", bufs=2) as cm,
            tc.tile_pool(name="cperbc", bufs=NB) as cperbc,
            tc.tile_pool(name="cph3", bufs=2) as cph3,
            tc.tile_pool(name="psum", bufs=1, space="PSUM") as psum_pool,
        ):
            # ---------------- constants ----------------
            # reversed class weights 11-c: argmax extracted via MAX of
            # eq*(11-c) -> smallest class index wins ties (= jnp.argmax).
            cio_i = consts.tile([128, C], I32, tag="cio_i")
            nc.gpsimd.iota(cio_i[:], pattern=[[-1, C]], base=C - 1,
                           channel_multiplier=0)
            cio = consts.tile([128, C], BF16, tag="cio")
            nc.vector.tensor_copy(cio[:], cio_i[:])

            jio_i = consts.tile([128, JW], I32, tag="jio_i")
            nc.gpsimd.iota(jio_i[:], pattern=[[1, JW]], base=1,
                           channel_multiplier=0)
            jio = consts.tile([128, JW], F32, tag="jio")
            nc.vector.tensor_copy(jio[:], jio_i[:])

            ones = consts.tile([128, 1], F32, tag="ones")
            nc.vector.memset(ones[:], 1.0)

            # ================= HOT PATH =================
            # lh layout (r, c, t): off = (r*C + c)*H + t
            lh = hot.tile([128, NB * C * H], F32, tag="lh")
            for r in range(NB):
                dst = _v(lh, r * C * H, [(H, C), (1, H)])
                src = lg[r * 128:(r + 1) * 128, :, 0:H]
                (nc.sync if r == 0 else nc.scalar).dma_start(dst, src)

            # ---- phase 1: exact argmax (DVE) ----
            m = hot.tile([128, NB * H], F32, tag="m")
            nc.vector.tensor_reduce(
                m[:], _v(lh, 0, [(C * H, NB), (1, H), (H, C)]),
                axis=AX, op=Alu.max)
            # eq layout (r, t, c): off = (r*H + t)*C + c  (c contiguous)
            eq = hot.tile([128, NB * H * C], BF16, tag="eq")
            nc.vector.tensor_tensor(
                _v(eq, 0, [(H * C, NB), (C, H), (1, C)]),
                _v(m, 0, [(H, NB), (1, H), (0, C)]),
                _v(lh, 0, [(C * H, NB), (1, H), (H, C)]), op=Alu.is_le)
            w = hot.tile([128, NB * H * C], BF16, tag="w")
            eq_v = _v(eq, 0, [(C, NB * H), (1, C)])
            w_v = _v(w, 0, [(C, NB * H), (1, C)])
            nc.vector.tensor_tensor(w_v, eq_v,
                                    _v(cio, 0, [(0, NB * H), (1, C)]),
                                    op=Alu.mult)
            # predsh[:, r*H + t] = 11 - argmax_c  (blank=11 -> 0)
            predsh = hot.tile([128, NB * H], BF16, tag="predsh")
            nc.vector.tensor_reduce(predsh[:], w_v, axis=AX, op=Alu.max)

            # ---- phase 2: dedup mask + cumsum (DVE) ----
            mask = hot.tile([128, NB * H], BF16, tag="mask")
            nc.vector.tensor_tensor(
                _v(mask, 1, [(H, NB), (1, H - 1)]),
                _v(predsh, 1, [(H, NB), (1, H - 1)]),
                _v(predsh, 0, [(H, NB), (1, H - 1)]), op=Alu.not_equal)
            # cols >= 1: mask &= (pred != blank);  col 0: mask = (pred != blank)
            nc.vector.scalar_tensor_tensor(
                _v(mask, 1, [(H, NB), (1, H - 1)]),
                _v(predsh, 1, [(H, NB), (1, H - 1)]), 0.0,
                _v(mask, 1, [(H, NB), (1, H - 1)]),
                op0=Alu.not_equal, op1=Alu.logical_and)
            nc.vector.tensor_scalar(
                _v(mask, 0, [(H, NB), (1, 1)]),
                _v(predsh, 0, [(H, NB), (1, 1)]), 0.0, None,
                op0=Alu.not_equal)
            pos1 = hot.tile([128, NB * H], F32, tag="pos1")
            for r in range(NB):
                sl = slice(r * H, (r + 1) * H)
                nc.vector.tensor_tensor_scan(
                    pos1[:, sl], mask[:, sl], mask[:, sl], 0.0,
                    op0=Alu.add, op1=Alu.max)

            # ---- flag (early, so idle engines can branch + prefire DMAs):
            # any row with pos1[H-1] < 6 needs the cold path
            rflag2 = hot.tile([128, NB], F32, tag="rflag2")
            nc.vector.tensor_scalar(rflag2[:],
                                    _v(pos1, H - 1, [(H, NB), (1, 1)]),
                                    float(MAXLEN), None, op0=Alu.is_lt)
            rflagr = hot.tile([128, 1], F32, tag="rflagr")
            nc.vector.tensor_reduce(rflagr[:], rflag2[:], axis=AX, op=Alu.add)
            fl_ps = psum_pool.tile([1, 1], F32, tag="fl_ps")
            nc.tensor.matmul(fl_ps[:], rflagr[:], ones[:], start=True, stop=True)
            fl_sb = hot.tile([1, 1], I32, tag="fl_sb")
            nc.vector.tensor_copy(fl_sb[:], fl_ps[:])
            fv = nc.values_load(fl_sb[:], min_val=0, max_val=257,
                                skip_runtime_bounds_check=True)

            # ---- phase 3a: entropy (Act: exp/ln; DVE: el, Z|S, H) ----
            # ee = [e | el], e = exp(l)  (no max-subtract: |l| <= ~6 is safe)
            ee = hot.tile([128, 2 * NB * C * H], F32, tag="ee")
            nc.scalar.activation(_v(ee, 0, [(1, NB * C * H)]), lh[:], Act.Exp)
            nc.vector.tensor_tensor(_v(ee, NB * C * H, [(1, NB * C * H)]),
                                    lh[:], _v(ee, 0, [(1, NB * C * H)]),
                                    op=Alu.mult)
            # ZS = [Z | S]: one fused reduce over c for both halves
            ZS = hot.tile([128, 2 * NB * H], F32, tag="ZS")
            nc.vector.tensor_reduce(
                ZS[:], _v(ee, 0, [(C * H, 2 * NB), (1, H), (H, C)]),
                axis=AX, op=Alu.add)
            Zv = _v(ZS, 0, [(1, NB * H)])
            Sv = _v(ZS, NB * H, [(1, NB * H)])
            lnZ = hot.tile([128, NB * H], F32, tag="lnZ")
            nc.scalar.activation(lnZ[:], Zv, Act.Ln)
            t1 = hot.tile([128, NB * H], F32, tag="t1")
            nc.vector.tensor_tensor(t1[:], Sv, Zv, op=Alu.divide)
            Ht = hot.tile([128, NB * H], F32, tag="Ht")
            nc.vector.tensor_tensor(Ht[:], lnZ[:], t1[:], op=Alu.subtract)

            # ---- phase 3b: slot extraction ----
            # rind = (pos1 == j+1) marks the kept position AND its trailing
            # run (repeats carry the same pred -> same u; blanks floor at
            # +16 < u_true), so u extracts via MAX without needing &mask.
            # cf needs the exact kept position: mm = mask*Ht, ADD-reduce.
            mm = hot.tile([128, NB * H], F32, tag="mm")
            nc.vector.tensor_tensor(mm[:], mask[:], Ht[:], op=Alu.mult)
            # rind layout (r, j, t): off = (r*JW + j)*H + t
            rind = hot.tile([128, NB * JW * H], F32, tag="rind")
            utmp = hot.tile([128, NB * JW * H], F32, tag="utmp")
            ctmp = hot.tile([128, NB * JW * H], F32, tag="ctmp")
            for r in range(NB):
                o = r * JW * H
                iv = _v(rind, o, [(H, JW), (1, H)])
                nc.vector.tensor_tensor(
                    iv, _v(pos1, r * H, [(0, JW), (1, H)]),
                    _v(jio, 0, [(1, JW), (0, H)]), op=Alu.is_equal)
                # utmp = ((11-pred) + 16) * rind
                nc.vector.scalar_tensor_tensor(
                    _v(utmp, o, [(H, JW), (1, H)]),
                    _v(predsh, r * H, [(0, JW), (1, H)]), 16.0, iv,
                    op0=Alu.add, op1=Alu.mult)
                nc.vector.tensor_tensor(
                    _v(ctmp, o, [(H, JW), (1, H)]), iv,
                    _v(mm, r * H, [(0, JW), (1, H)]), op=Alu.mult)

            u_acc = hot.tile([128, NB * JW], F32, tag="u_acc")
            cf_acc = hot.tile([128, NB * JW], F32, tag="cf_acc")
            rdim = [(H, NB * JW), (1, H)]
            nc.vector.tensor_reduce(u_acc[:], _v(utmp, 0, rdim), axis=AX,
                                    op=Alu.max)
            nc.vector.tensor_reduce(cf_acc[:], _v(ctmp, 0, rdim), axis=AX,
                                    op=Alu.add)

            # ================= COLD PATH (worst-case guard) =================
            # Statistically never taken: full-T recompute of preds/mask/pos1,
            # then accumulate slot contributions from t >= H into the accs.
            with tc.If(fv >= 1):
                TcC = 256
                predsC_b, maskC_b, pos1C_b = [], [], []
                for bc in range(NB):
                    b0 = bc * 128
                    predsC = cperbc.tile([128, T], BF16, tag="predsC")
                    for k in range(T // TcC):
                        t0 = k * TcC
                        lt = clt.tile([128, C * TcC], F32, tag="lt")
                        lt_ct = _v(lt, 0, [(TcC, C), (1, TcC)])
                        lt_tc = _v(lt, 0, [(1, TcC), (TcC, C)])
                        nc.sync.dma_start(lt_ct, lg[b0:b0 + 128, :, t0:t0 + TcC])
                        mC = cm.tile([128, TcC], F32, tag="mC")
                        nc.vector.tensor_reduce(mC[:], lt_tc, axis=AX, op=Alu.max)
                        eqC = ceq.tile([128, C * TcC], BF16, tag="eqC")
                        eq_tc = _v(eqC, 0, [(C, TcC), (1, C)])
                        m_bc = _v(mC, 0, [(1, TcC), (0, C)])
                        nc.vector.scalar_tensor_tensor(
                            eq_tc, m_bc, 1.0, lt_tc, op0=Alu.mult, op1=Alu.is_le)
                        wC = ceq.tile([128, C * TcC], BF16, tag="wC")
                        w_tc = _v(wC, 0, [(C, TcC), (1, C)])
                        cio_bc = _v(cio, 0, [(0, TcC), (1, C)])
                        nc.vector.tensor_tensor(w_tc, eq_tc, cio_bc, op=Alu.mult)
                        nc.vector.tensor_reduce(predsC[:, t0:t0 + TcC], w_tc,
                                                axis=AX, op=Alu.max)
                    maskC = cperbc.tile([128, T], BF16, tag="maskC")
                    nc.vector.memset(maskC[:, 0:1], 1.0)
                    nc.vector.tensor_tensor(maskC[:, 1:T], predsC[:, 1:T],
                                            predsC[:, 0:T - 1], op=Alu.not_equal)
                    nc.vector.scalar_tensor_tensor(
                        maskC[:], predsC[:], 0.0, maskC[:],
                        op0=Alu.not_equal, op1=Alu.logical_and)
                    pos1C = cperbc.tile([128, T], F32, tag="pos1C")
                    nc.vector.tensor_tensor_scan(
                        pos1C[:], maskC[:], maskC[:], 0.0,
                        op0=Alu.add, op1=Alu.max)
                    predsC_b.append(predsC)
                    maskC_b.append(maskC)
                    pos1C_b.append(pos1C)

                for bc in range(NB):
                    b0 = bc * 128
                    asl = slice(bc * JW, (bc + 1) * JW)
                    for Sc in range(H, T, 128):
                        Ec = min(Sc + 128, T)
                        sz = Ec - Sc
                        lh3 = cph3.tile([128, C * sz], F32, tag="lh3")
                        nc.gpsimd.dma_start(_v(lh3, 0, [(sz, C), (1, sz)]),
                                            lg[b0:b0 + 128, :, Sc:Ec])
                        e3 = cph3.tile([128, C * sz], F32, tag="e3")
                        nc.scalar.activation(e3[:], lh3[:], Act.Exp)
                        el3 = cph3.tile([128, C * sz], F32, tag="el3")
                        nc.vector.tensor_tensor(el3[:], lh3[:], e3[:], op=Alu.mult)
                        Z3 = cph3.tile([128, sz], F32, tag="Z3")
                        nc.vector.tensor_reduce(Z3[:], _v(e3, 0, [(1, sz), (sz, C)]),
                                                axis=AX, op=Alu.add)
                        S3 = cph3.tile([128, sz], F32, tag="S3")
                        nc.vector.tensor_reduce(S3[:], _v(el3, 0, [(1, sz), (sz, C)]),
                                                axis=AX, op=Alu.add)
                        lnZ3 = cph3.tile([128, sz], F32, tag="lnZ3")
                        nc.scalar.activation(lnZ3[:], Z3[:], Act.Ln)
                        t13 = cph3.tile([128, sz], F32, tag="t13")
                        nc.vector.tensor_tensor(t13[:], S3[:], Z3[:], op=Alu.divide)
                        Ht3 = cph3.tile([128, sz], F32, tag="Ht3")
                        nc.vector.tensor_tensor(Ht3[:], lnZ3[:], t13[:],
                                                op=Alu.subtract)

                        pos1C, maskC, predsC = pos1C_b[bc], maskC_b[bc], predsC_b[bc]
                        p1s = _v(pos1C, Sc, [(0, JW), (1, sz)])
                        msks = _v(maskC, Sc, [(0, JW), (1, sz)])
                        prds = _v(predsC, Sc, [(0, JW), (1, sz)])
                        jio_bc2 = _v(jio, 0, [(1, JW), (0, sz)])
                        ind3 = cph3.tile([128, JW * sz], F32, tag="ind3")
                        ind3_v = _v(ind3, 0, [(sz, JW), (1, sz)])
                        nc.vector.tensor_tensor(ind3_v, p1s, jio_bc2,
                                                op=Alu.is_equal)
                        nc.vector.tensor_tensor(ind3_v, ind3_v, msks,
                                                op=Alu.logical_and)

                        tmp3 = cph3.tile([128, JW * sz], F32, tag="tmp3")
                        tmp3_v = _v(tmp3, 0, [(sz, JW), (1, sz)])
                        red = cph3.tile([128, JW], F32, tag="red")
                        nc.vector.scalar_tensor_tensor(
                            tmp3_v, prds, 16.0, ind3_v,
                            op0=Alu.add, op1=Alu.mult)
                        nc.vector.tensor_reduce(red[:], tmp3_v, axis=AX, op=Alu.add)
                        # hot u_acc is MAX-encoded; strict-ind chunk sums are
                        # u_true-or-0, so max-combine is exact
                        nc.vector.tensor_tensor(u_acc[:, asl], u_acc[:, asl],
                                                red[:], op=Alu.max)
                        Ht3_bv = _v(Ht3, 0, [(0, JW), (1, sz)])
                        nc.vector.tensor_tensor(tmp3_v, ind3_v, Ht3_bv, op=Alu.mult)
                        red3 = cph3.tile([128, JW], F32, tag="red3")
                        nc.vector.tensor_reduce(red3[:], tmp3_v, axis=AX, op=Alu.add)
                        nc.vector.tensor_tensor(cf_acc[:, asl], cf_acc[:, asl],
                                                red3[:], op=Alu.add)

            # ==================== finalize + output ====================
            # u = (11-pred) + 16 for a filled slot, 0 for empty.
            # dec = (28*(u>0) - 1) - u   (filled -> pred; empty -> -1)
            decf = hot.tile([128, NB * JW], F32, tag="decf")
            nc.vector.tensor_scalar(decf[:], u_acc[:], 0.0, 28.0,
                                    op0=Alu.is_gt, op1=Alu.mult)
            nc.vector.scalar_tensor_tensor(decf[:], decf[:], -1.0, u_acc[:],
                                           op0=Alu.add, op1=Alu.subtract)
            deci = hot.tile([128, NB * JW], I32, tag="deci")
            nc.vector.tensor_copy(deci[:], decf[:])

            # one DMA per output tensor: SBUF (r,j) -> DRAM both row-halves
            dap = dec_o[0:128, :]
            nc.sync.dma_start(
                bass.AP(dap.tensor, dap.offset,
                        [dap.ap[0], [128 * JW, NB], [1, JW]]), deci[:])
            cap = conf_o[0:128, :]
            nc.scalar.dma_start(
                bass.AP(cap.tensor, cap.offset,
                        [cap.ap[0], [128 * JW, NB], [1, JW]]), cf_acc[:])

    return nc


_CACHED = {}


def _get_program(B, T, head=HEAD):
    key = (B, T, head)
    if key not in _CACHED:
        nc = bacc.Bacc()
        build_decoder(nc, B, T, head=head)
        nc.compile()
        _CACHED[key] = nc
    return _CACHED[key]


def kernel(logits: np.ndarray):
    logits = np.ascontiguousarray(logits, dtype=np.float32)
    B, c, T = logits.shape
    assert c == C
    Bs = B // N_CORES
    nc = _get_program(Bs, T)
    in_maps = [
        {"logits": logits[i * Bs:(i + 1) * Bs]} for i in range(N_CORES)
    ]
    res = run_bass_kernel_spmd(nc, in_maps, core_ids=list(range(N_CORES)))
    dec = np.concatenate([r["decoded"] for r in res.results], axis=0)
    conf = np.concatenate([r["confidences"] for r in res.results], axis=0)
    return dec.astype(np.int32), conf.astype(np.float32)
